# revision 1
# baseline (speedup 1.0000x reference)
"""Llama4TextExperts MoE grouped-GEMM kernel for 8 Trainium2 NeuronCores.

Expert-parallel: core e owns expert e and the pre-sorted token block
hidden_states[e*4096:(e+1)*4096]. No collectives needed.

Per-core pipeline (all dims multiples of 128):
  x (4096, 2048) --PE transpose--> xT chunks [H on partitions]
  mm1: gate_upT = W1_block.T @ xT  (float32r matmuls, PSUM fp32 accum)
  SwiGLU: actT = silu(gate) * up   (ACT silu + DVE mul, written as f32r)
  mm2: out = actT.T @ W2_slice     (natural [token, H] layout out of PSUM)
  store via ACT copy -> SBUF -> DMA (2KB-contiguous rows)

float32r runs the PE at 1 cycle/row (4x over fp32) with ~1e-4 relative
error. Walrus requires every SBUF operand of an f32r matmul to be
*produced* with dtype float32r: weights get it from the DMA (bitcast),
xT/actT from the DVE copy/mul outputs.
"""

import numpy as np

try:
    import concourse.bass as bass  # noqa: F401
except ImportError:
    import sys

    sys.path.insert(0, "/opt/trn_rl_repo")

import concourse.mybir as mybir
import concourse.tile as tile
from concourse import bacc
from concourse.bass_utils import run_bass_kernel_spmd
from concourse.masks import make_identity

F32 = mybir.dt.float32
F32R = mybir.dt.float32r
SILU = mybir.ActivationFunctionType.Silu
P = 128

NCORES = 8
H_FULL = 2048  # hidden size
D_FULL = 2048  # expert intermediate size
T_TOTAL = 32768
T_CORE = T_TOTAL // NCORES  # 4096 tokens per expert/core


def emit_moe(nc, out_ap, x_ap, w1_ap, w2_ap, T, H, D, TC):
    """Emit the per-core MoE program. T tokens, chunked by TC."""
    K1 = H // P  # contraction tiles for mm1
    MP = D // P  # gate/up column-block pairs
    K2 = D // P  # contraction tiles for mm2
    HS = 256  # mm2 moving-operand width (>=256 keeps f32r at full rate)
    NH = H // HS
    NT = TC // P  # token tiles per chunk
    MMW = 512  # mm1 moving-operand width
    NHALF = TC // MMW
    NCH = T // TC

    with tile.TileContext(nc) as tc:
        with (
            tc.tile_pool(name="const", bufs=1) as constp,
            tc.tile_pool(name="xnat", bufs=2) as xnatp,
            tc.tile_pool(name="xT", bufs=1) as xTp,
            tc.tile_pool(name="actT", bufs=1) as actTp,
            tc.tile_pool(name="w1", bufs=2) as w1p,
            tc.tile_pool(name="w2", bufs=2) as w2p,
            tc.tile_pool(name="sil", bufs=2) as silp,
            tc.tile_pool(name="ost", bufs=4) as ostp,
            tc.tile_pool(name="psT", bufs=2, space="PSUM") as psTp,
            tc.tile_pool(name="psg", bufs=2, space="PSUM") as psgp,
            tc.tile_pool(name="psu", bufs=2, space="PSUM") as psup,
            tc.tile_pool(name="ps2", bufs=2, space="PSUM") as ps2p,
        ):
            ident = constp.tile([P, P], F32)
            make_identity(nc, ident)

            for c in range(NCH):
                t0 = c * TC

                # ---- load x chunk and transpose to [H-part, token] ----
                xT = xTp.tile([P, K1 * TC], F32R, tag="xT")
                for tt in range(NT):
                    xn = xnatp.tile([P, H], F32, tag="xn")
                    nc.sync.dma_start(
                        out=xn[:], in_=x_ap[t0 + tt * P : t0 + (tt + 1) * P, :]
                    )
                    for k in range(K1):
                        pst = psTp.tile([P, P], F32, tag="psT")
                        nc.tensor.transpose(pst[:], xn[:, k * P : (k + 1) * P], ident[:])
                        nc.vector.tensor_copy(
                            xT[:, k * TC + tt * P : k * TC + (tt + 1) * P], pst[:]
                        )

                # ---- mm1 + SwiGLU -> actT ----
                actT = actTp.tile([P, K2 * TC], F32R, tag="actT")
                for mp in range(MP):
                    w1g = w1p.tile([P, K1 * P], F32R, tag="w1")
                    nc.sync.dma_start(
                        out=w1g[:].rearrange("p (k c) -> p k c", k=K1),
                        in_=w1_ap[:, mp * P : (mp + 1) * P]
                        .bitcast(F32R)
                        .rearrange("(k p) c -> p k c", p=P),
                    )
                    w1u = w1p.tile([P, K1 * P], F32R, tag="w1")
                    nc.sync.dma_start(
                        out=w1u[:].rearrange("p (k c) -> p k c", k=K1),
                        in_=w1_ap[:, D + mp * P : D + (mp + 1) * P]
                        .bitcast(F32R)
                        .rearrange("(k p) c -> p k c", p=P),
                    )
                    for hf in range(NHALF):
                        off = hf * MMW
                        psg = psgp.tile([P, MMW], F32, tag="psg")
                        for k in range(K1):
                            nc.tensor.matmul(
                                psg[:],
                                w1g[:, k * P : (k + 1) * P],
                                xT[:, k * TC + off : k * TC + off + MMW],
                                start=(k == 0),
                                stop=(k == K1 - 1),
                            )
                        sil = silp.tile([P, MMW], F32, tag="sil")
                        nc.scalar.activation(sil[:], psg[:], SILU)
                        psu = psup.tile([P, MMW], F32, tag="psu")
                        for k in range(K1):
                            nc.tensor.matmul(
                                psu[:],
                                w1u[:, k * P : (k + 1) * P],
                                xT[:, k * TC + off : k * TC + off + MMW],
                                start=(k == 0),
                                stop=(k == K1 - 1),
                            )
                        nc.vector.tensor_mul(
                            actT[:, mp * TC + off : mp * TC + off + MMW],
                            sil[:],
                            psu[:],
                        )

                # ---- mm2 -> natural-layout output ----
                for h in range(NH):
                    w2s = w2p.tile([P, K2 * HS], F32R, tag="w2")
                    nc.sync.dma_start(
                        out=w2s[:].rearrange("p (k c) -> p k c", k=K2),
                        in_=w2_ap[:, h * HS : (h + 1) * HS]
                        .bitcast(F32R)
                        .rearrange("(k p) c -> p k c", p=P),
                    )
                    for tt in range(NT):
                        ps2 = ps2p.tile([P, HS], F32, tag="ps2")
                        for k2 in range(K2):
                            nc.tensor.matmul(
                                ps2[:],
                                actT[:, k2 * TC + tt * P : k2 * TC + (tt + 1) * P],
                                w2s[:, k2 * HS : (k2 + 1) * HS],
                                start=(k2 == 0),
                                stop=(k2 == K2 - 1),
                            )
                        ost = ostp.tile([P, HS], F32, tag="ost")
                        nc.scalar.copy(ost[:], ps2[:])
                        nc.sync.dma_start(
                            out=out_ap[
                                t0 + tt * P : t0 + (tt + 1) * P,
                                h * HS : (h + 1) * HS,
                            ],
                            in_=ost[:],
                        )


def build(T=T_CORE, H=H_FULL, D=D_FULL, TC=1024):
    nc = bacc.Bacc("TRN2", target_bir_lowering=False, debug=False)
    x = nc.dram_tensor("x", [T, H], F32, kind="ExternalInput").ap()
    w1 = nc.dram_tensor("w1", [H, 2 * D], F32, kind="ExternalInput").ap()
    w2 = nc.dram_tensor("w2", [D, H], F32, kind="ExternalInput").ap()
    out = nc.dram_tensor("out", [T, H], F32, kind="ExternalOutput").ap()
    emit_moe(nc, out, x, w1, w2, T, H, D, TC)
    nc.compile()
    return nc


_NC_CACHE = {}


def _get_nc():
    if "nc" not in _NC_CACHE:
        _NC_CACHE["nc"] = build()
    return _NC_CACHE["nc"]


def run_sharded(hidden_states, gate_up_proj, down_proj, trace=False, **kwargs):
    """Run on 8 cores; returns (full_output, BassKernelResults)."""
    hidden_states = np.ascontiguousarray(np.asarray(hidden_states, dtype=np.float32))
    gate_up_proj = np.ascontiguousarray(np.asarray(gate_up_proj, dtype=np.float32))
    down_proj = np.ascontiguousarray(np.asarray(down_proj, dtype=np.float32))

    nc = _get_nc()
    in_maps = [
        {
            "x": hidden_states[e * T_CORE : (e + 1) * T_CORE],
            "w1": gate_up_proj[e],
            "w2": down_proj[e],
        }
        for e in range(NCORES)
    ]
    res = run_bass_kernel_spmd(
        nc, in_maps, core_ids=list(range(NCORES)), trace=trace, **kwargs
    )
    out = np.concatenate([res.results[e]["out"] for e in range(NCORES)], axis=0)
    return out, res


def kernel(hidden_states, gate_up_proj, down_proj):
    out, _ = run_sharded(hidden_states, gate_up_proj, down_proj)
    return out


# revision 2
# speedup vs baseline: 1.1048x; 1.1048x over previous
"""Llama4TextExperts MoE grouped-GEMM kernel for 8 Trainium2 NeuronCores.

Expert-parallel: core e owns expert e and the pre-sorted token block
hidden_states[e*4096:(e+1)*4096]. No collectives needed.

Per-core pipeline (all dims multiples of 128):
  x (4096, 2048) --PE transpose--> xT chunks [H on partitions]
  mm1: gate_upT = W1_block.T @ xT  (float32r matmuls, PSUM fp32 accum)
  SwiGLU: actT = silu(gate) * up   (ACT silu + DVE mul, written as f32r)
  mm2: out = actT.T @ W2_slice     (natural [token, H] layout out of PSUM)
  store via ACT copy -> SBUF -> DMA (2KB-contiguous rows)

float32r runs the PE at 1 cycle/row (4x over fp32) with ~1e-4 relative
error. Walrus requires every SBUF operand of an f32r matmul to be
*produced* with dtype float32r: weights get it from the DMA (bitcast),
xT/actT from the DVE copy/mul outputs.
"""

import numpy as np

try:
    import concourse.bass as bass  # noqa: F401
except ImportError:
    import sys

    sys.path.insert(0, "/opt/trn_rl_repo")

import concourse.mybir as mybir
import concourse.tile as tile
from concourse import bacc
from concourse.bass_utils import run_bass_kernel_spmd
from concourse.masks import make_identity

F32 = mybir.dt.float32
F32R = mybir.dt.float32r
SILU = mybir.ActivationFunctionType.Silu
P = 128

NCORES = 8
H_FULL = 2048  # hidden size
D_FULL = 2048  # expert intermediate size
T_TOTAL = 32768
T_CORE = T_TOTAL // NCORES  # 4096 tokens per expert/core


def emit_moe(nc, out_ap, x_ap, w1_ap, w2_ap, T, H, D, TC):
    """Emit the per-core MoE program. T tokens, chunked by TC."""
    K1 = H // P  # contraction tiles for mm1
    KH = K1 // 2  # half-block k-tiles (weights stream as half blocks)
    MP = D // P  # gate/up column-block pairs
    K2 = D // P  # contraction tiles for mm2
    K2H = K2 // 2
    M2 = H // P  # mm2 output column blocks
    NT = TC // P  # token tiles per chunk
    MMW = 512  # moving-operand width (f32r full-rate needs >=256; LDW hidden at 512)
    NHALF = TC // MMW
    NCH = T // TC

    def load_w_halves(pool, w_ap, rows, col0, kh, tag):
        """Load [rows x 128] weight block as two half-K tiles (better prefetch)."""
        tiles = []
        for hlf in range(2):
            t = pool.tile([P, kh * P], F32R, tag=tag)
            nc.sync.dma_start(
                out=t[:].rearrange("p (k c) -> p k c", k=kh),
                in_=w_ap[hlf * (rows // 2) : (hlf + 1) * (rows // 2), col0 : col0 + P]
                .bitcast(F32R)
                .rearrange("(k p) c -> p k c", p=P),
            )
            tiles.append(t)
        return tiles

    with tile.TileContext(nc) as tc:
        with (
            tc.tile_pool(name="const", bufs=1) as constp,
            tc.tile_pool(name="xnat", bufs=2) as xnatp,
            tc.tile_pool(name="xT", bufs=1) as xTp,
            tc.tile_pool(name="actT", bufs=1) as actTp,
            tc.tile_pool(name="w1", bufs=6) as w1p,
            tc.tile_pool(name="w2", bufs=6) as w2p,
            tc.tile_pool(name="sil", bufs=2) as silp,
            tc.tile_pool(name="o2s", bufs=2) as o2sp,
            tc.tile_pool(name="ost", bufs=4) as ostp,
            tc.tile_pool(name="psT", bufs=2, space="PSUM") as psTp,
            tc.tile_pool(name="psg", bufs=2, space="PSUM") as psgp,
            tc.tile_pool(name="psu", bufs=2, space="PSUM") as psup,
            tc.tile_pool(name="ps2", bufs=2, space="PSUM") as ps2p,
        ):
            ident = constp.tile([P, P], F32)
            make_identity(nc, ident)

            # transpose-backs deferred one MM group so PE never waits on the
            # DVE evacuation of the PSUM tile they read
            pending = []

            def flush_pending():
                while pending:
                    o2s, dst_rows, col0 = pending.pop(0)
                    for tb in range(4):
                        pst = psTp.tile([P, P], F32, tag="psT")
                        nc.tensor.transpose(
                            pst[:], o2s[:, tb * P : (tb + 1) * P], ident[:]
                        )
                        ost = ostp.tile([P, P], F32, tag="ost")
                        if tb % 2 == 0:
                            nc.scalar.copy(ost[:], pst[:])
                        else:
                            nc.vector.tensor_copy(ost[:], pst[:])
                        nc.sync.dma_start(
                            out=out_ap[
                                dst_rows + tb * P : dst_rows + (tb + 1) * P,
                                col0 : col0 + P,
                            ],
                            in_=ost[:],
                        )

            for c in range(NCH):
                t0 = c * TC

                # ---- load x chunk and transpose to [H-part, token] ----
                xT = xTp.tile([P, K1 * TC], F32R, tag="xT")
                for tt in range(NT):
                    xn = xnatp.tile([P, H], F32, tag="xn")
                    nc.sync.dma_start(
                        out=xn[:], in_=x_ap[t0 + tt * P : t0 + (tt + 1) * P, :]
                    )
                    for k in range(K1):
                        pst = psTp.tile([P, P], F32, tag="psT")
                        nc.tensor.transpose(pst[:], xn[:, k * P : (k + 1) * P], ident[:])
                        nc.vector.tensor_copy(
                            xT[:, k * TC + tt * P : k * TC + (tt + 1) * P], pst[:]
                        )

                # ---- mm1 + SwiGLU -> actT ----
                actT = actTp.tile([P, K2 * TC], F32R, tag="actT")
                for mp in range(MP):
                    w1g = load_w_halves(w1p, w1_ap, H, mp * P, KH, "w1")
                    w1u = load_w_halves(w1p, w1_ap, H, D + mp * P, KH, "w1")
                    for hf in range(NHALF):
                        off = hf * MMW
                        psg = psgp.tile([P, MMW], F32, tag="psg")
                        for k in range(K1):
                            nc.tensor.matmul(
                                psg[:],
                                w1g[k // KH][:, (k % KH) * P : (k % KH + 1) * P],
                                xT[:, k * TC + off : k * TC + off + MMW],
                                start=(k == 0),
                                stop=(k == K1 - 1),
                            )
                        sil = silp.tile([P, MMW], F32, tag="sil")
                        nc.scalar.activation(sil[:], psg[:], SILU)
                        psu = psup.tile([P, MMW], F32, tag="psu")
                        for k in range(K1):
                            nc.tensor.matmul(
                                psu[:],
                                w1u[k // KH][:, (k % KH) * P : (k % KH + 1) * P],
                                xT[:, k * TC + off : k * TC + off + MMW],
                                start=(k == 0),
                                stop=(k == K1 - 1),
                            )
                        nc.vector.tensor_mul(
                            actT[:, mp * TC + off : mp * TC + off + MMW],
                            sil[:],
                            psu[:],
                        )

                # ---- mm2: W2 stationary, actT moving -> transposed PSUM,
                # ---- PE transpose-back to natural layout ----
                for m2 in range(M2):
                    w2b = load_w_halves(w2p, w2_ap, D, m2 * P, K2H, "w2")
                    for hf in range(NHALF):
                        off = hf * MMW
                        ps2 = ps2p.tile([P, MMW], F32, tag="ps2")
                        for k2 in range(K2):
                            nc.tensor.matmul(
                                ps2[:],
                                w2b[k2 // K2H][:, (k2 % K2H) * P : (k2 % K2H + 1) * P],
                                actT[:, k2 * TC + off : k2 * TC + off + MMW],
                                start=(k2 == 0),
                                stop=(k2 == K2 - 1),
                            )
                        o2s = o2sp.tile([P, MMW], F32, tag="o2s")
                        nc.vector.tensor_copy(o2s[:], ps2[:])
                        flush_pending()
                        pending.append((o2s, t0 + off, m2 * P))
                flush_pending()


def build(T=T_CORE, H=H_FULL, D=D_FULL, TC=1024):
    nc = bacc.Bacc("TRN2", target_bir_lowering=False, debug=False)
    x = nc.dram_tensor("x", [T, H], F32, kind="ExternalInput").ap()
    w1 = nc.dram_tensor("w1", [H, 2 * D], F32, kind="ExternalInput").ap()
    w2 = nc.dram_tensor("w2", [D, H], F32, kind="ExternalInput").ap()
    out = nc.dram_tensor("out", [T, H], F32, kind="ExternalOutput").ap()
    emit_moe(nc, out, x, w1, w2, T, H, D, TC)
    nc.compile()
    return nc


_NC_CACHE = {}


def _get_nc():
    if "nc" not in _NC_CACHE:
        _NC_CACHE["nc"] = build()
    return _NC_CACHE["nc"]


def run_sharded(hidden_states, gate_up_proj, down_proj, trace=False, **kwargs):
    """Run on 8 cores; returns (full_output, BassKernelResults)."""
    hidden_states = np.ascontiguousarray(np.asarray(hidden_states, dtype=np.float32))
    gate_up_proj = np.ascontiguousarray(np.asarray(gate_up_proj, dtype=np.float32))
    down_proj = np.ascontiguousarray(np.asarray(down_proj, dtype=np.float32))

    nc = _get_nc()
    in_maps = [
        {
            "x": hidden_states[e * T_CORE : (e + 1) * T_CORE],
            "w1": gate_up_proj[e],
            "w2": down_proj[e],
        }
        for e in range(NCORES)
    ]
    res = run_bass_kernel_spmd(
        nc, in_maps, core_ids=list(range(NCORES)), trace=trace, **kwargs
    )
    out = np.concatenate([res.results[e]["out"] for e in range(NCORES)], axis=0)
    return out, res


def kernel(hidden_states, gate_up_proj, down_proj):
    out, _ = run_sharded(hidden_states, gate_up_proj, down_proj)
    return out


# revision 6
# speedup vs baseline: 1.1901x; 1.0771x over previous
"""Llama4TextExperts MoE grouped-GEMM kernel for 8 Trainium2 NeuronCores.

Expert-parallel: core e owns expert e and the pre-sorted token block
hidden_states[e*4096:(e+1)*4096]. No collectives needed.

Per-core pipeline (all dims multiples of 128):
  x (4096, 2048) --PE transpose--> xT chunks [H on partitions]
  mm1: gate_upT = W1_block.T @ xT  (float32r matmuls, PSUM fp32 accum)
  SwiGLU: actT = silu(gate) * up   (ACT silu + DVE mul, written as f32r)
  mm2: out = actT.T @ W2_slice     (natural [token, H] layout out of PSUM)
  store via ACT copy -> SBUF -> DMA (2KB-contiguous rows)

float32r runs the PE at 1 cycle/row (4x over fp32) with ~1e-4 relative
error. Walrus requires every SBUF operand of an f32r matmul to be
*produced* with dtype float32r: weights get it from the DMA (bitcast),
xT/actT from the DVE copy/mul outputs.
"""

import numpy as np

try:
    import concourse.bass as bass  # noqa: F401
except ImportError:
    import sys

    sys.path.insert(0, "/opt/trn_rl_repo")

import concourse.mybir as mybir
import concourse.tile as tile
from concourse import bacc
from concourse.bass_utils import run_bass_kernel_spmd
from concourse.masks import make_identity

F32 = mybir.dt.float32
F32R = mybir.dt.float32r
SILU = mybir.ActivationFunctionType.Silu
P = 128

NCORES = 8
H_FULL = 2048  # hidden size
D_FULL = 2048  # expert intermediate size
T_TOTAL = 32768
T_CORE = T_TOTAL // NCORES  # 4096 tokens per expert/core


def emit_moe(nc, out_ap, x_ap, w1_ap, w2_ap, T, H, D, TC):
    """Emit the per-core MoE program. T tokens, chunked by TC."""
    K1 = H // P  # contraction tiles for mm1
    KH = K1 // 2  # half-block k-tiles (weights stream as half blocks)
    MP = D // P  # gate/up column-block pairs
    K2 = D // P  # contraction tiles for mm2
    K2H = K2 // 2
    M2 = H // P  # mm2 output column blocks
    NT = TC // P  # token tiles per chunk
    MMW = 512  # moving-operand width (f32r full-rate needs >=256; LDW hidden at 512)
    NHALF = TC // MMW
    NCH = T // TC

    def load_w_halves(pool, w_ap, rows, col0, kh, tag):
        """Load [rows x 128] weight block as two half-K tiles (better prefetch)."""
        tiles = []
        for hlf in range(2):
            t = pool.tile([P, kh * P], F32R, tag=tag)
            nc.sync.dma_start(
                out=t[:].rearrange("p (k c) -> p k c", k=kh),
                in_=w_ap[hlf * (rows // 2) : (hlf + 1) * (rows // 2), col0 : col0 + P]
                .bitcast(F32R)
                .rearrange("(k p) c -> p k c", p=P),
            )
            tiles.append(t)
        return tiles

    with tile.TileContext(nc) as tc:
        with (
            tc.tile_pool(name="const", bufs=1) as constp,
            tc.tile_pool(name="xnat", bufs=2) as xnatp,
            tc.tile_pool(name="xT", bufs=1) as xTp,
            tc.tile_pool(name="actT", bufs=1) as actTp,
            tc.tile_pool(name="w1", bufs=6) as w1p,
            tc.tile_pool(name="w2", bufs=6) as w2p,
            tc.tile_pool(name="sil", bufs=2) as silp,
            tc.tile_pool(name="o2s", bufs=2) as o2sp,
            tc.tile_pool(name="ost", bufs=4) as ostp,
            tc.tile_pool(name="psX", bufs=2, space="PSUM") as psXp,
            tc.tile_pool(name="psTo", bufs=2, space="PSUM") as psTop,
            tc.tile_pool(name="psg", bufs=1, space="PSUM") as psgp,
            tc.tile_pool(name="psu", bufs=1, space="PSUM") as psup,
            tc.tile_pool(name="ps2", bufs=2, space="PSUM") as ps2p,
        ):
            ident = constp.tile([P, P], F32)
            make_identity(nc, ident)

            # transpose-backs deferred one MM group so PE never waits on the
            # DVE evacuation of the PSUM tile they read
            pending = []

            def flush_pending():
                while pending:
                    o2s, dst_rows, col0 = pending.pop(0)
                    for tb in range(4):
                        pst = psTop.tile([P, P], F32, tag="psTo")
                        nc.tensor.transpose(
                            pst[:], o2s[:, tb * P : (tb + 1) * P], ident[:]
                        )
                        ost = ostp.tile([P, P], F32, tag="ost")
                        nc.scalar.copy(ost[:], pst[:])
                        nc.sync.dma_start(
                            out=out_ap[
                                dst_rows + tb * P : dst_rows + (tb + 1) * P,
                                col0 : col0 + P,
                            ],
                            in_=ost[:],
                        )

            # x load+transpose for chunk c, emitted in 2*NT steps of KH
            # transposes each so they can interleave with mm2 MM groups of
            # the previous chunk (keeps HAM warm; transposes alone don't)
            xstate = {}

            def x_step(c, i):
                tt, half = i // 2, i % 2
                t0c = c * TC
                if half == 0:
                    if tt == 0:
                        xstate[c] = {
                            "xT": xTp.tile(
                                [P, K1 * TC], F32R, tag="xT", name=f"xT_{c}"
                            )
                        }
                    xn = xnatp.tile([P, H], F32, tag="xn", name=f"xn_{c}_{tt}")
                    nc.sync.dma_start(
                        out=xn[:], in_=x_ap[t0c + tt * P : t0c + (tt + 1) * P, :]
                    )
                    xstate[c]["xn"] = xn
                xn = xstate[c]["xn"]
                xT = xstate[c]["xT"]
                for k in range(half * KH, (half + 1) * KH):
                    pst = psXp.tile([P, P], F32, tag="psX")
                    nc.tensor.transpose(pst[:], xn[:, k * P : (k + 1) * P], ident[:])
                    nc.vector.tensor_copy(
                        xT[:, k * TC + tt * P : k * TC + (tt + 1) * P], pst[:]
                    )

            for c in range(NCH):
                t0 = c * TC

                if c == 0:
                    for i in range(2 * NT):
                        x_step(0, i)
                xT = xstate[c]["xT"]

                # ---- mm1 + SwiGLU -> actT ----
                actT = actTp.tile([P, K2 * TC], F32R, tag="actT")
                for mp in range(MP):
                    w1g = load_w_halves(w1p, w1_ap, H, mp * P, KH, "w1")
                    w1u = load_w_halves(w1p, w1_ap, H, D + mp * P, KH, "w1")
                    for hf in range(NHALF):
                        off = hf * MMW
                        psg = psgp.tile([P, MMW], F32, tag="psg")
                        for k in range(K1):
                            nc.tensor.matmul(
                                psg[:],
                                w1g[k // KH][:, (k % KH) * P : (k % KH + 1) * P],
                                xT[:, k * TC + off : k * TC + off + MMW],
                                start=(k == 0),
                                stop=(k == K1 - 1),
                            )
                        sil = silp.tile([P, MMW], F32, tag="sil")
                        nc.scalar.activation(sil[:], psg[:], SILU)
                        psu = psup.tile([P, MMW], F32, tag="psu")
                        for k in range(K1):
                            nc.tensor.matmul(
                                psu[:],
                                w1u[k // KH][:, (k % KH) * P : (k % KH + 1) * P],
                                xT[:, k * TC + off : k * TC + off + MMW],
                                start=(k == 0),
                                stop=(k == K1 - 1),
                            )
                        nc.vector.tensor_mul(
                            actT[:, mp * TC + off : mp * TC + off + MMW],
                            sil[:],
                            psu[:],
                        )

                # ---- mm2: W2 stationary, actT moving -> transposed PSUM,
                # ---- PE transpose-back to natural layout. Next chunk's x
                # ---- transposes are threaded between m2 blocks. ----
                for m2 in range(M2):
                    w2b = load_w_halves(w2p, w2_ap, D, m2 * P, K2H, "w2")
                    for hf in range(NHALF):
                        off = hf * MMW
                        ps2 = ps2p.tile([P, MMW], F32, tag="ps2")
                        for k2 in range(K2):
                            nc.tensor.matmul(
                                ps2[:],
                                w2b[k2 // K2H][:, (k2 % K2H) * P : (k2 % K2H + 1) * P],
                                actT[:, k2 * TC + off : k2 * TC + off + MMW],
                                start=(k2 == 0),
                                stop=(k2 == K2 - 1),
                            )
                        o2s = o2sp.tile([P, MMW], F32, tag="o2s")
                        nc.vector.tensor_copy(o2s[:], ps2[:])
                        flush_pending()
                        pending.append((o2s, t0 + off, m2 * P))
                    if c + 1 < NCH and m2 < 2 * NT:
                        x_step(c + 1, m2)
                if c + 1 < NCH:
                    for i in range(min(M2, 2 * NT), 2 * NT):
                        x_step(c + 1, i)
                flush_pending()


def build(T=T_CORE, H=H_FULL, D=D_FULL, TC=1024):
    nc = bacc.Bacc("TRN2", target_bir_lowering=False, debug=False)
    x = nc.dram_tensor("x", [T, H], F32, kind="ExternalInput").ap()
    w1 = nc.dram_tensor("w1", [H, 2 * D], F32, kind="ExternalInput").ap()
    w2 = nc.dram_tensor("w2", [D, H], F32, kind="ExternalInput").ap()
    out = nc.dram_tensor("out", [T, H], F32, kind="ExternalOutput").ap()
    emit_moe(nc, out, x, w1, w2, T, H, D, TC)
    nc.compile()
    return nc


_NC_CACHE = {}


def _get_nc():
    if "nc" not in _NC_CACHE:
        _NC_CACHE["nc"] = build()
    return _NC_CACHE["nc"]


def run_sharded(hidden_states, gate_up_proj, down_proj, trace=False, **kwargs):
    """Run on 8 cores; returns (full_output, BassKernelResults)."""
    hidden_states = np.ascontiguousarray(np.asarray(hidden_states, dtype=np.float32))
    gate_up_proj = np.ascontiguousarray(np.asarray(gate_up_proj, dtype=np.float32))
    down_proj = np.ascontiguousarray(np.asarray(down_proj, dtype=np.float32))

    nc = _get_nc()
    in_maps = [
        {
            "x": hidden_states[e * T_CORE : (e + 1) * T_CORE],
            "w1": gate_up_proj[e],
            "w2": down_proj[e],
        }
        for e in range(NCORES)
    ]
    res = run_bass_kernel_spmd(
        nc, in_maps, core_ids=list(range(NCORES)), trace=trace, **kwargs
    )
    out = np.concatenate([res.results[e]["out"] for e in range(NCORES)], axis=0)
    return out, res


def kernel(hidden_states, gate_up_proj, down_proj):
    out, _ = run_sharded(hidden_states, gate_up_proj, down_proj)
    return out


# revision 9
# speedup vs baseline: 1.2068x; 1.0140x over previous
"""Llama4TextExperts MoE grouped-GEMM kernel for 8 Trainium2 NeuronCores.

Expert-parallel: core e owns expert e and the pre-sorted token block
hidden_states[e*4096:(e+1)*4096]. No collectives needed.

Per-core pipeline (all dims multiples of 128):
  x (4096, 2048) --PE transpose--> xT chunks [H on partitions]
  mm1: gate_upT = W1_block.T @ xT  (float32r matmuls, PSUM fp32 accum)
  SwiGLU: actT = silu(gate) * up   (ACT silu + DVE mul, written as f32r)
  mm2: out = actT.T @ W2_slice     (natural [token, H] layout out of PSUM)
  store via ACT copy -> SBUF -> DMA (2KB-contiguous rows)

float32r runs the PE at 1 cycle/row (4x over fp32) with ~1e-4 relative
error. Walrus requires every SBUF operand of an f32r matmul to be
*produced* with dtype float32r: weights get it from the DMA (bitcast),
xT/actT from the DVE copy/mul outputs.
"""

import numpy as np

try:
    import concourse.bass as bass  # noqa: F401
except ImportError:
    import sys

    sys.path.insert(0, "/opt/trn_rl_repo")

import concourse.mybir as mybir
import concourse.tile as tile
from concourse import bacc
from concourse.bass_utils import run_bass_kernel_spmd
from concourse.masks import make_identity

F32 = mybir.dt.float32
F32R = mybir.dt.float32r
SILU = mybir.ActivationFunctionType.Silu
P = 128

NCORES = 8
H_FULL = 2048  # hidden size
D_FULL = 2048  # expert intermediate size
T_TOTAL = 32768
T_CORE = T_TOTAL // NCORES  # 4096 tokens per expert/core


def emit_moe(nc, out_ap, x_ap, w1_ap, w2_ap, T, H, D, TC):
    """Emit the per-core MoE program. T tokens, chunked by TC."""
    K1 = H // P  # contraction tiles for mm1
    KH = K1 // 2  # half-block k-tiles (weights stream as half blocks)
    MP = D // P  # gate/up column-block pairs
    K2 = D // P  # contraction tiles for mm2
    K2H = K2 // 2
    M2 = H // P  # mm2 output column blocks
    NT = TC // P  # token tiles per chunk
    MMW = 512  # moving-operand width (f32r full-rate needs >=256; LDW hidden at 512)
    NHALF = TC // MMW
    NCH = T // TC

    def load_w_halves(pool, w_ap, rows, col0, kh, tag):
        """Load [rows x 128] weight block as two half-K tiles (better prefetch)."""
        tiles = []
        for hlf in range(2):
            t = pool.tile([P, kh * P], F32R, tag=tag)
            nc.sync.dma_start(
                out=t[:].rearrange("p (k c) -> p k c", k=kh),
                in_=w_ap[hlf * (rows // 2) : (hlf + 1) * (rows // 2), col0 : col0 + P]
                .bitcast(F32R)
                .rearrange("(k p) c -> p k c", p=P),
            )
            tiles.append(t)
        return tiles

    with tile.TileContext(nc) as tc:
        with (
            tc.tile_pool(name="const", bufs=1) as constp,
            tc.tile_pool(name="xnat", bufs=2) as xnatp,
            tc.tile_pool(name="xT", bufs=1) as xTp,
            tc.tile_pool(name="actT", bufs=1) as actTp,
            tc.tile_pool(name="w1", bufs=6) as w1p,
            tc.tile_pool(name="w2", bufs=6) as w2p,
            tc.tile_pool(name="sil", bufs=2) as silp,
            tc.tile_pool(name="o2s", bufs=3) as o2sp,
            tc.tile_pool(name="ost", bufs=4) as ostp,
            tc.tile_pool(name="psX", bufs=2, space="PSUM") as psXp,
            tc.tile_pool(name="psTo", bufs=2, space="PSUM") as psTop,
            tc.tile_pool(name="psg", bufs=1, space="PSUM") as psgp,
            tc.tile_pool(name="psu", bufs=1, space="PSUM") as psup,
            tc.tile_pool(name="ps2", bufs=2, space="PSUM") as ps2p,
        ):
            ident = constp.tile([P, P], F32)
            make_identity(nc, ident)

            # transpose-backs deferred one MM group so PE never waits on the
            # DVE evacuation of the PSUM tile they read
            pending = []

            def flush_pending(keep=0):
                while len(pending) > keep:
                    o2s, dst_rows, col0 = pending.pop(0)
                    for tb in range(4):
                        pst = psTop.tile([P, P], F32, tag="psTo")
                        nc.tensor.transpose(
                            pst[:], o2s[:, tb * P : (tb + 1) * P], ident[:]
                        )
                        ost = ostp.tile([P, P], F32, tag="ost")
                        nc.scalar.copy(ost[:], pst[:])
                        nc.sync.dma_start(
                            out=out_ap[
                                dst_rows + tb * P : dst_rows + (tb + 1) * P,
                                col0 : col0 + P,
                            ],
                            in_=ost[:],
                        )

            # x load+transpose for chunk c, emitted in 2*NT steps of KH
            # transposes each so they can interleave with mm2 MM groups of
            # the previous chunk (keeps HAM warm; transposes alone don't)
            xstate = {}

            def x_step(c, i):
                tt, half = i // 2, i % 2
                t0c = c * TC
                if half == 0:
                    if tt == 0:
                        xstate[c] = {
                            "xT": xTp.tile(
                                [P, K1 * TC], F32R, tag="xT", name=f"xT_{c}"
                            )
                        }
                    xn = xnatp.tile([P, H], F32, tag="xn", name=f"xn_{c}_{tt}")
                    nc.sync.dma_start(
                        out=xn[:], in_=x_ap[t0c + tt * P : t0c + (tt + 1) * P, :]
                    )
                    xstate[c]["xn"] = xn
                xn = xstate[c]["xn"]
                xT = xstate[c]["xT"]
                for k in range(half * KH, (half + 1) * KH):
                    pst = psXp.tile([P, P], F32, tag="psX")
                    nc.tensor.transpose(pst[:], xn[:, k * P : (k + 1) * P], ident[:])
                    nc.vector.tensor_copy(
                        xT[:, k * TC + tt * P : k * TC + (tt + 1) * P], pst[:]
                    )

            for c in range(NCH):
                t0 = c * TC

                if c == 0:
                    for i in range(2 * NT):
                        x_step(0, i)
                xT = xstate[c]["xT"]

                # ---- mm1 + SwiGLU -> actT ----
                actT = actTp.tile([P, K2 * TC], F32R, tag="actT")
                for mp in range(MP):
                    w1g = load_w_halves(w1p, w1_ap, H, mp * P, KH, "w1")
                    w1u = load_w_halves(w1p, w1_ap, H, D + mp * P, KH, "w1")
                    for hf in range(NHALF):
                        off = hf * MMW
                        psg = psgp.tile([P, MMW], F32, tag="psg")
                        for k in range(K1):
                            nc.tensor.matmul(
                                psg[:],
                                w1g[k // KH][:, (k % KH) * P : (k % KH + 1) * P],
                                xT[:, k * TC + off : k * TC + off + MMW],
                                start=(k == 0),
                                stop=(k == K1 - 1),
                            )
                        sil = silp.tile([P, MMW], F32, tag="sil")
                        nc.scalar.activation(sil[:], psg[:], SILU)
                        psu = psup.tile([P, MMW], F32, tag="psu")
                        for k in range(K1):
                            nc.tensor.matmul(
                                psu[:],
                                w1u[k // KH][:, (k % KH) * P : (k % KH + 1) * P],
                                xT[:, k * TC + off : k * TC + off + MMW],
                                start=(k == 0),
                                stop=(k == K1 - 1),
                            )
                        nc.vector.tensor_mul(
                            actT[:, mp * TC + off : mp * TC + off + MMW],
                            sil[:],
                            psu[:],
                        )

                # ---- mm2: W2 stationary, actT moving -> transposed PSUM,
                # ---- PE transpose-back to natural layout. Next chunk's x
                # ---- transposes are threaded between m2 blocks. ----
                for m2 in range(M2):
                    w2b = load_w_halves(w2p, w2_ap, D, m2 * P, K2H, "w2")
                    for hf in range(NHALF):
                        off = hf * MMW
                        ps2 = ps2p.tile([P, MMW], F32, tag="ps2")
                        for k2 in range(K2):
                            nc.tensor.matmul(
                                ps2[:],
                                w2b[k2 // K2H][:, (k2 % K2H) * P : (k2 % K2H + 1) * P],
                                actT[:, k2 * TC + off : k2 * TC + off + MMW],
                                start=(k2 == 0),
                                stop=(k2 == K2 - 1),
                            )
                        o2s = o2sp.tile([P, MMW], F32, tag="o2s")
                        nc.vector.tensor_copy(o2s[:], ps2[:])
                        flush_pending(keep=1)
                        pending.append((o2s, t0 + off, m2 * P))
                    if c + 1 < NCH and m2 < 2 * NT:
                        x_step(c + 1, m2)
                if c + 1 < NCH:
                    for i in range(min(M2, 2 * NT), 2 * NT):
                        x_step(c + 1, i)
                flush_pending()


def build(T=T_CORE, H=H_FULL, D=D_FULL, TC=1024):
    nc = bacc.Bacc("TRN2", target_bir_lowering=False, debug=False)
    x = nc.dram_tensor("x", [T, H], F32, kind="ExternalInput").ap()
    w1 = nc.dram_tensor("w1", [H, 2 * D], F32, kind="ExternalInput").ap()
    w2 = nc.dram_tensor("w2", [D, H], F32, kind="ExternalInput").ap()
    out = nc.dram_tensor("out", [T, H], F32, kind="ExternalOutput").ap()
    emit_moe(nc, out, x, w1, w2, T, H, D, TC)
    nc.compile()
    return nc


_NC_CACHE = {}


def _get_nc():
    if "nc" not in _NC_CACHE:
        _NC_CACHE["nc"] = build()
    return _NC_CACHE["nc"]


def run_sharded(hidden_states, gate_up_proj, down_proj, trace=False, **kwargs):
    """Run on 8 cores; returns (full_output, BassKernelResults)."""
    hidden_states = np.ascontiguousarray(np.asarray(hidden_states, dtype=np.float32))
    gate_up_proj = np.ascontiguousarray(np.asarray(gate_up_proj, dtype=np.float32))
    down_proj = np.ascontiguousarray(np.asarray(down_proj, dtype=np.float32))

    nc = _get_nc()
    in_maps = [
        {
            "x": hidden_states[e * T_CORE : (e + 1) * T_CORE],
            "w1": gate_up_proj[e],
            "w2": down_proj[e],
        }
        for e in range(NCORES)
    ]
    res = run_bass_kernel_spmd(
        nc, in_maps, core_ids=list(range(NCORES)), trace=trace, **kwargs
    )
    out = np.concatenate([res.results[e]["out"] for e in range(NCORES)], axis=0)
    return out, res


def kernel(hidden_states, gate_up_proj, down_proj):
    out, _ = run_sharded(hidden_states, gate_up_proj, down_proj)
    return out


# revision 13
# speedup vs baseline: 1.2151x; 1.0069x over previous
"""Llama4TextExperts MoE grouped-GEMM kernel for 8 Trainium2 NeuronCores.

Expert-parallel: core e owns expert e and the pre-sorted token block
hidden_states[e*4096:(e+1)*4096]. No collectives needed.

Per-core pipeline (all dims multiples of 128):
  x (4096, 2048) --PE transpose--> xT chunks [H on partitions]
  mm1: gate_upT = W1_block.T @ xT  (float32r matmuls, PSUM fp32 accum)
  SwiGLU: actT = silu(gate) * up   (ACT silu + DVE mul, written as f32r)
  mm2: out = actT.T @ W2_slice     (natural [token, H] layout out of PSUM)
  store via ACT copy -> SBUF -> DMA (2KB-contiguous rows)

float32r runs the PE at 1 cycle/row (4x over fp32) with ~1e-4 relative
error. Walrus requires every SBUF operand of an f32r matmul to be
*produced* with dtype float32r: weights get it from the DMA (bitcast),
xT/actT from the DVE copy/mul outputs.
"""

import numpy as np

try:
    import concourse.bass as bass  # noqa: F401
except ImportError:
    import sys

    sys.path.insert(0, "/opt/trn_rl_repo")

import concourse.mybir as mybir
import concourse.tile as tile
from concourse import bacc
from concourse.bass_utils import run_bass_kernel_spmd
from concourse.masks import make_identity

F32 = mybir.dt.float32
F32R = mybir.dt.float32r
SILU = mybir.ActivationFunctionType.Silu
P = 128

NCORES = 8
H_FULL = 2048  # hidden size
D_FULL = 2048  # expert intermediate size
T_TOTAL = 32768
T_CORE = T_TOTAL // NCORES  # 4096 tokens per expert/core


def emit_moe(nc, out_ap, x_ap, w1_ap, w2_ap, T, H, D, TC):
    """Emit the per-core MoE program. T tokens, chunked by TC."""
    K1 = H // P  # contraction tiles for mm1
    KH = K1 // 2  # half-block k-tiles (weights stream as half blocks)
    MP = D // P  # gate/up column-block pairs
    K2 = D // P  # contraction tiles for mm2
    K2H = K2 // 2
    M2 = H // P  # mm2 output column blocks
    NT = TC // P  # token tiles per chunk
    MMW = 512  # moving-operand width (f32r full-rate needs >=256; LDW hidden at 512)
    NHALF = TC // MMW
    NCH = T // TC

    def load_w_halves(pool, w_ap, rows, col0, kh, tag):
        """Load [rows x 128] weight block as two half-K tiles (better prefetch)."""
        tiles = []
        for hlf in range(2):
            t = pool.tile([P, kh * P], F32R, tag=tag)
            nc.sync.dma_start(
                out=t[:].rearrange("p (k c) -> p k c", k=kh),
                in_=w_ap[hlf * (rows // 2) : (hlf + 1) * (rows // 2), col0 : col0 + P]
                .bitcast(F32R)
                .rearrange("(k p) c -> p k c", p=P),
            )
            tiles.append(t)
        return tiles

    with tile.TileContext(nc) as tc:
        with (
            tc.tile_pool(name="const", bufs=1) as constp,
            tc.tile_pool(name="xnat", bufs=2) as xnatp,
            tc.tile_pool(name="xT", bufs=1) as xTp,
            tc.tile_pool(name="actT", bufs=1) as actTp,
            tc.tile_pool(name="w1", bufs=6) as w1p,
            tc.tile_pool(name="w2", bufs=6) as w2p,
            tc.tile_pool(name="sil", bufs=2) as silp,
            tc.tile_pool(name="o2s", bufs=3) as o2sp,
            tc.tile_pool(name="ost", bufs=4) as ostp,
            tc.tile_pool(name="psX", bufs=2, space="PSUM") as psXp,
            tc.tile_pool(name="psTo", bufs=2, space="PSUM") as psTop,
            tc.tile_pool(name="psg", bufs=1, space="PSUM") as psgp,
            tc.tile_pool(name="psu", bufs=1, space="PSUM") as psup,
            tc.tile_pool(name="ps2", bufs=2, space="PSUM") as ps2p,
        ):
            ident = constp.tile([P, P], F32)
            make_identity(nc, ident)

            # transpose-backs deferred one MM group so PE never waits on the
            # DVE evacuation of the PSUM tile they read
            pending = []

            def flush_pending(keep=0):
                while len(pending) > keep:
                    o2s, dst_rows, col0 = pending.pop(0)
                    for tb in range(4):
                        pst = psTop.tile([P, P], F32, tag="psTo")
                        nc.tensor.transpose(
                            pst[:], o2s[:, tb * P : (tb + 1) * P], ident[:]
                        )
                        ost = ostp.tile([P, P], F32, tag="ost")
                        nc.scalar.copy(ost[:], pst[:])
                        nc.sync.dma_start(
                            out=out_ap[
                                dst_rows + tb * P : dst_rows + (tb + 1) * P,
                                col0 : col0 + P,
                            ],
                            in_=ost[:],
                        )

            # x load+transpose for chunk c, emitted in 2*NT steps of KH
            # transposes each so they can interleave with mm2 MM groups of
            # the previous chunk (keeps HAM warm; transposes alone don't)
            xstate = {}

            def xn_load(c, tt):
                if tt >= NT:
                    return
                t0c = c * TC
                if tt == 0:
                    xstate[c] = {
                        "xT": xTp.tile([P, K1 * TC], F32R, tag="xT", name=f"xT_{c}"),
                        "xn": {},
                    }
                xn = xnatp.tile([P, H], F32, tag="xn", name=f"xn_{c}_{tt}")
                nc.sync.dma_start(
                    out=xn[:], in_=x_ap[t0c + tt * P : t0c + (tt + 1) * P, :]
                )
                xstate[c]["xn"][tt] = xn

            def x_step(c, i):
                tt, half = i // 2, i % 2
                xn = xstate[c]["xn"][tt]
                xT = xstate[c]["xT"]
                for k in range(half * KH, (half + 1) * KH):
                    pst = psXp.tile([P, P], F32, tag="psX")
                    nc.tensor.transpose(pst[:], xn[:, k * P : (k + 1) * P], ident[:])
                    nc.vector.tensor_copy(
                        xT[:, k * TC + tt * P : k * TC + (tt + 1) * P], pst[:]
                    )
                if half == 1:
                    xstate[c]["xn"].pop(tt)
                    xn_load(c, tt + 1)

            for c in range(NCH):
                t0 = c * TC

                if c == 0:
                    xn_load(0, 0)
                    for i in range(2 * NT):
                        x_step(0, i)
                xT = xstate[c]["xT"]

                # ---- mm1 + SwiGLU -> actT ----
                actT = actTp.tile([P, K2 * TC], F32R, tag="actT")
                for mp in range(MP):
                    w1g = load_w_halves(w1p, w1_ap, H, mp * P, KH, "w1")
                    w1u = load_w_halves(w1p, w1_ap, H, D + mp * P, KH, "w1")
                    for hf in range(NHALF):
                        off = hf * MMW
                        psg = psgp.tile([P, MMW], F32, tag="psg")
                        for k in range(K1):
                            nc.tensor.matmul(
                                psg[:],
                                w1g[k // KH][:, (k % KH) * P : (k % KH + 1) * P],
                                xT[:, k * TC + off : k * TC + off + MMW],
                                start=(k == 0),
                                stop=(k == K1 - 1),
                            )
                        sil = silp.tile([P, MMW], F32, tag="sil")
                        nc.scalar.activation(sil[:], psg[:], SILU)
                        psu = psup.tile([P, MMW], F32, tag="psu")
                        for k in range(K1):
                            nc.tensor.matmul(
                                psu[:],
                                w1u[k // KH][:, (k % KH) * P : (k % KH + 1) * P],
                                xT[:, k * TC + off : k * TC + off + MMW],
                                start=(k == 0),
                                stop=(k == K1 - 1),
                            )
                        nc.vector.tensor_mul(
                            actT[:, mp * TC + off : mp * TC + off + MMW],
                            sil[:],
                            psu[:],
                        )

                # ---- mm2: W2 stationary, actT moving -> transposed PSUM,
                # ---- PE transpose-back to natural layout. Next chunk's x
                # ---- transposes are threaded between m2 blocks. ----
                for m2 in range(M2):
                    w2b = load_w_halves(w2p, w2_ap, D, m2 * P, K2H, "w2")
                    for hf in range(NHALF):
                        off = hf * MMW
                        ps2 = ps2p.tile([P, MMW], F32, tag="ps2")
                        for k2 in range(K2):
                            nc.tensor.matmul(
                                ps2[:],
                                w2b[k2 // K2H][:, (k2 % K2H) * P : (k2 % K2H + 1) * P],
                                actT[:, k2 * TC + off : k2 * TC + off + MMW],
                                start=(k2 == 0),
                                stop=(k2 == K2 - 1),
                            )
                        o2s = o2sp.tile([P, MMW], F32, tag="o2s")
                        nc.vector.tensor_copy(o2s[:], ps2[:])
                        flush_pending(keep=1)
                        pending.append((o2s, t0 + off, m2 * P))
                    if c + 1 < NCH and m2 < 2 * NT:
                        if m2 == 0:
                            xn_load(c + 1, 0)
                        x_step(c + 1, m2)
                if c + 1 < NCH:
                    for i in range(min(M2, 2 * NT), 2 * NT):
                        x_step(c + 1, i)
                flush_pending()


def build(T=T_CORE, H=H_FULL, D=D_FULL, TC=1024):
    nc = bacc.Bacc("TRN2", target_bir_lowering=False, debug=False)
    x = nc.dram_tensor("x", [T, H], F32, kind="ExternalInput").ap()
    w1 = nc.dram_tensor("w1", [H, 2 * D], F32, kind="ExternalInput").ap()
    w2 = nc.dram_tensor("w2", [D, H], F32, kind="ExternalInput").ap()
    out = nc.dram_tensor("out", [T, H], F32, kind="ExternalOutput").ap()
    emit_moe(nc, out, x, w1, w2, T, H, D, TC)
    nc.compile()
    return nc


_NC_CACHE = {}


def _get_nc():
    if "nc" not in _NC_CACHE:
        _NC_CACHE["nc"] = build()
    return _NC_CACHE["nc"]


def run_sharded(hidden_states, gate_up_proj, down_proj, trace=False, **kwargs):
    """Run on 8 cores; returns (full_output, BassKernelResults)."""
    hidden_states = np.ascontiguousarray(np.asarray(hidden_states, dtype=np.float32))
    gate_up_proj = np.ascontiguousarray(np.asarray(gate_up_proj, dtype=np.float32))
    down_proj = np.ascontiguousarray(np.asarray(down_proj, dtype=np.float32))

    nc = _get_nc()
    in_maps = [
        {
            "x": hidden_states[e * T_CORE : (e + 1) * T_CORE],
            "w1": gate_up_proj[e],
            "w2": down_proj[e],
        }
        for e in range(NCORES)
    ]
    res = run_bass_kernel_spmd(
        nc, in_maps, core_ids=list(range(NCORES)), trace=trace, **kwargs
    )
    out = np.concatenate([res.results[e]["out"] for e in range(NCORES)], axis=0)
    return out, res


def kernel(hidden_states, gate_up_proj, down_proj):
    import os

    # The NTFF trace path needs antenv.axon_hooks, absent in this image;
    # make sure a stray BASS_TRACE env can't route us into it.
    os.environ["BASS_NEVER_TRACE"] = "1"
    try:
        out, _ = run_sharded(hidden_states, gate_up_proj, down_proj)
    finally:
        del os.environ["BASS_NEVER_TRACE"]
    return out
